# revision 30
# baseline (speedup 1.0000x reference)
"""Trainium2 Bass kernel for the NEUROPULS unitary NxN photonic mesh.

Parallel-scan reformulation. The reference chain is 128 sequential
structured steps X <- CR@MMI@diag(p_it)@MMI@X (last step without CR),
starting from X = diag(p_0) and finishing with a diag(p_129) row scale;
the output is the accumulated 256x256 complex matrix.

Instead of running 128 latency-bound steps on every core, each core m
computes the *group product* G_m = A_{16m+16}...A_{16m+1} of its 16
structured factors as a band-packed matrix (band +-32, 76 stored
diagonals, fp32), using the same E-step/CR-step pair-layout machinery as
the direct method -- G starts as (packed) identity, so a group step costs
the same as a direct step but on a ~65-wide band instead of 32 columns.
diag(p_0) folds into core 0's G init, diag(p_129) into core 7's
post-scale, and the missing final crossing is an identity-CR via per-step
blend masks, so the SPMD program is uniform across cores.

The 8 packed G's are AllGather'ed (69.6KB/core, fp16), scattered into
zero-backed DRAM strips, and densified for free by reading them back with
a skewed access pattern (stride 511 over rows): row r's band lands at
dense columns [r-32, r+32], everything else reads pre-zeroed margin.
Each core then redundantly computes F^T = G_0^T G_1^T ... G_7^T as two
independent 4-multiply half-chains that interleave on PE/DVE (lhsT = G_k
row-major, rhs = running product; PSUM accumulation over row blocks and
the complex cross terms, with a pre-negated imaginary weight plane built
on GpSimd), joined by a PE transpose of the left half and a final
multiply. The host transposes F^T into the full output.
"""

import numpy as np

import bass_rust
import concourse.bass as bass
import concourse.mybir as mybir
import concourse.tile as tile
from concourse.bass_utils import run_bass_kernel_spmd

N = 256
NCORES = 8
GITS = 16          # iterations per group
J = 80             # packed band width (diag offsets; 80 keeps fp16 planes 16B-aligned)
W = 40             # packed center: G[r, j] = G_dense[r, r + j - W]
JB = 65            # shipped band slice: packed j in [W-32, W+32]
WARMN = 76         # PE warm-up matmuls bridging the collective
SP = 512           # strip pitch (elements) in the zero-backed skew DRAM
S0 = 224           # strip start offset within a row's pitch

IL_MMI = 0.02
IMB = 0.01
IL_CR = 0.02
CT = 0.01

_A_MMI = float(np.sqrt(1.0 - IL_MMI))
AT = _A_MMI * float(np.sqrt((1.0 + IMB) / 2.0))
AR = _A_MMI * float(np.sqrt((1.0 - IMB) / 2.0))
_A_CR = float(np.sqrt(1.0 - IL_CR))
G1S = _A_CR * float(np.sqrt(CT))        # CR diag (mid rows)
G2C = _A_CR * float(np.sqrt(1.0 - CT))  # CR off-diag (x i); also thru

F32 = mybir.dt.float32
F16 = mybir.dt.float16
MULT = mybir.AluOpType.mult
ADD = mybir.AluOpType.add
SUB = mybir.AluOpType.subtract
SIN = mybir.ActivationFunctionType.Sin
PI = float(np.pi)

_ENGINE_SEM_PREFIXES = {
    "DVE": ("DVE_",),
    "ACT": ("ACT_", "Activation_"),
    "PE": ("PE_",),
    "POOL": ("Pool_", "POOL_"),
    "SP": ("SP_",),
}


def strip_same_engine_waits(nc):
    for bb in nc.main_func.blocks:
        for ins in bb.instructions:
            si = getattr(ins, "sync_info", None)
            if si is None:
                continue
            eng = getattr(ins, "engine", None)
            pres = _ENGINE_SEM_PREFIXES.get(getattr(eng, "name", ""), ())
            if not pres:
                continue
            kept = [
                w
                for w in si.on_wait
                if not (
                    w.sync_type == "semaphore"
                    and w.ant_name
                    and w.ant_name.startswith(pres)
                )
            ]
            if len(kept) != len(si.on_wait):
                si.on_wait = kept
                ins.sync_info = si


def split_multi_waits(nc):
    """This walrus build allows one sync-wait per instruction: hoist extra
    waits onto same-engine Drain nops inserted just before the instruction."""
    for bb in nc.main_func.blocks:
        insts = bb.instructions
        i = 0
        while i < len(insts):
            ins = insts[i]
            si = getattr(ins, "sync_info", None)
            if si is None or len(si.on_wait) <= 1:
                i += 1
                continue
            waits = list(si.on_wait)
            for k, w in enumerate(waits[:-1]):
                d = mybir.InstDrain(name=f"{ins.name}_waitsplit{k}", ins=[], outs=[])
                d.engine = ins.engine
                d.sync_info = bass_rust.SyncInfo(on_wait=[w], on_update=[])
                insts.insert(i, d)
                i += 1
            si.on_wait = [waits[-1]]
            ins.sync_info = si
            i += 1


def fix_sync_waits(nc):
    split_multi_waits(nc)


def _skew_ap(strips, k, rb):
    """Dense row-major read of row-block rb of G_k (both planes) from the
    zero-backed strip area: element (r, c) of plane pl at strip offset
    r*SP + S0 + (c - r + W).  Dims: [r-part 128, pl 2, c 256]."""
    ap = strips[:]
    base = k * (2 * 256 * SP) + rb * 128 * (SP - 1) + (S0 + W)
    ap.ap = bass_rust.VecI64Pair([[SP - 1, 128], [256 * SP, 2], [1, 256]])
    ap.offset = base
    return ap


def build_nc(nsteps=GITS):
    nc = bass.Bass(num_devices=8)

    thg = nc.dram_tensor("thg", [18, N], F32, kind="ExternalInput")
    isg0 = nc.dram_tensor("isg0", [128, 1], F32, kind="ExternalInput")
    postm = nc.dram_tensor("postm", [128, 1], F32, kind="ExternalInput")
    gescd = nc.dram_tensor("gescd", [128, GITS, 2], F32, kind="ExternalInput")
    wconst = nc.dram_tensor("wconst", [4, 128, 128], F32, kind="ExternalInput")
    wconstL = nc.dram_tensor("wconstL", [4, 128, 128], F32, kind="ExternalInput")
    permw = nc.dram_tensor("permw", [4, 128, 128], F32, kind="ExternalInput")
    seld = nc.dram_tensor("seld", [128, 2, 32], F16, kind="ExternalInput")
    out_d = nc.dram_tensor("out", [128, 2, 64], F16, kind="ExternalOutput")

    gsend = nc.dram_tensor("gsend", [256, 2, JB], F16, kind="Internal")
    gall = nc.dram_tensor("gall", [8, 256, 2, JB], F16, kind="Internal")
    strips = nc.dram_tensor("strips", [8, 2, 256, SP], F16, kind="Internal")

    with tile.TileContext(nc) as tc:
        with (
            tc.tile_pool(name="coef", bufs=1) as cp,
            tc.tile_pool(name="state", bufs=1) as sp,
            tc.tile_pool(name="mchain", bufs=2) as mp,
            tc.tile_pool(name="lts", bufs=2) as lp,
            tc.tile_pool(name="psum", bufs=2, space="PSUM") as pp,
        ):
            # ---------------- setup: trig + step coefficients ----------------
            th = cp.tile([128, 18, 2], F32, tag="th")
            Ct = cp.tile([128, 18, 2], F32, tag="Ct")
            St = cp.tile([128, 18, 2], F32, tag="St")
            wrk = cp.tile([128, 18, 2], F32, tag="wrk")
            wrp = cp.tile([128, 18, 2], F32, tag="wrp")
            zb = cp.tile([128, 1], F32, tag="zb")
            d1r = cp.tile([128, GITS, 2], F32, tag="d1r")
            d1i = cp.tile([128, GITS, 2], F32, tag="d1i")
            d2r = cp.tile([128, GITS, 2], F32, tag="d2r")
            d2i = cp.tile([128, GITS, 2], F32, tag="d2i")
            isg = cp.tile([128, 1], F32, tag="isg")
            psm = cp.tile([128, 1], F32, tag="psm")
            gt = cp.tile([128, GITS, 2], F32, tag="gt")
            Wt = cp.tile([128, 4, 128], F32, tag="Wt")
            WtL = cp.tile([128, 4, 128], F32, tag="WtL")
            Pt = cp.tile([128, 4, 128], F32, tag="Pt")
            wrm = cp.tile([128, 2, 128], F32, tag="wrm")
            selt = cp.tile([128, 2, 32], F16, tag="selt")

            nc.sync.dma_start(th[:], thg[:].rearrange("it (k e) -> k it e", k=128, e=2))
            nc.sync.dma_start(isg[:], isg0[:])
            nc.sync.dma_start(psm[:], postm[:])
            nc.sync.dma_start(gt[:], gescd[:])
            nc.sync.dma_start(Wt[:], wconst[:].rearrange("w p f -> p w f"))
            nc.sync.dma_start(WtL[:], wconstL[:].rearrange("w p f -> p w f"))
            nc.sync.dma_start(Pt[:], permw[:].rearrange("w p f -> p w f"))
            nc.sync.dma_start(selt[:], seld[:])
            nc.vector.memset(zb[:], 0.0)
            nc.vector.memset(wrm[:], 0.5)

            # zero-fill the skew strips (after the inputs so they don't
            # delay setup; overlaps phase A on the DMA engines)
            zt = cp.tile([128, 2048], F16, tag="zt")
            nc.vector.memset(zt[:], 0.0)
            for g in range(8):
                dst = strips[g].rearrange(
                    "pl (pa pb) c -> (pl pa) (pb c)", pa=64, pb=4
                )
                nc.gpsimd.dma_start(dst, zt[:])

            # sin/cos with range reduction into (-pi, pi]
            nc.vector.tensor_scalar(wrp[:], th[:], PI, -2 * PI, mybir.AluOpType.is_gt, MULT)
            nc.vector.tensor_tensor(wrk[:], th[:], wrp[:], ADD)
            nc.scalar.activation(St[:], wrk[:], SIN, bias=zb[:])
            nc.vector.tensor_scalar(wrk[:], th[:], PI / 2, None, ADD)
            nc.vector.tensor_scalar(wrp[:], wrk[:], PI, -2 * PI, mybir.AluOpType.is_gt, MULT)
            nc.vector.tensor_tensor(wrk[:], wrk[:], wrp[:], ADD)
            nc.scalar.activation(Ct[:], wrk[:], SIN, bias=zb[:])

            Cmid = Ct[:, :GITS, :]
            Smid = St[:, :GITS, :]
            Csw = Ct[:, :GITS, ::-1]
            Ssw = St[:, :GITS, ::-1]
            wmid = wrk[:, :GITS, :]

            # d1 = at^2 p - ar^2 p^sigma ; d2 = i at ar (p + p^sigma)
            nc.vector.tensor_scalar(wmid, Csw, -AR * AR, None, MULT)
            nc.vector.scalar_tensor_tensor(d1r[:], Cmid, AT * AT, wmid, MULT, ADD)
            nc.vector.tensor_scalar(wmid, Ssw, -AR * AR, None, MULT)
            nc.vector.scalar_tensor_tensor(d1i[:], Smid, AT * AT, wmid, MULT, ADD)
            nc.vector.tensor_tensor(wmid, Smid, Ssw, ADD)
            nc.vector.tensor_scalar(d2r[:], wmid, -AT * AR, None, MULT)
            nc.vector.tensor_tensor(wmid, Cmid, Csw, ADD)
            nc.vector.tensor_scalar(d2i[:], wmid, AT * AR, None, MULT)

            # p129 post-scale blend: ceff = postm*c129 + (1-postm); seff = postm*s129
            ceff = cp.tile([128, 2], F32, tag="ceff")
            seff = cp.tile([128, 2], F32, tag="seff")
            seffN = cp.tile([128, 2], F32, tag="seffN")
            npsm = cp.tile([128, 1], F32, tag="npsm")
            nc.vector.tensor_scalar(npsm[:], psm[:], -1.0, 1.0, MULT, ADD)
            for e in range(2):
                nc.vector.scalar_tensor_tensor(
                    ceff[:, e : e + 1], Ct[:, 17, e : e + 1], psm[:], npsm[:], MULT, ADD
                )
                nc.vector.tensor_scalar(seff[:, e : e + 1], St[:, 17, e : e + 1], psm[:], None, MULT)
            nc.vector.tensor_scalar(seffN[:], seff[:], -1.0, None, MULT)

            # ---------------- phase A state (fp16 band) ----------------
            Gpp_a = sp.tile([128, 2, 2, J], F32, tag="Ga")
            Gpp_b = sp.tile([128, 2, 2, J], F32, tag="Gb")
            Gpp = [Gpp_a, Gpp_b]
            G = Gpp[0]
            Vpp_a = sp.tile([128, 2, 2, J], F32, tag="Va")
            Vpp_b = sp.tile([128, 2, 2, J], F32, tag="Vb")
            Vpp = [Vpp_a, Vpp_b]
            V = Vpp[0]
            Ypp_a = sp.tile([128, 2, 2, J], F32, tag="Ya")
            Ypp_b = sp.tile([128, 2, 2, J], F32, tag="Yb")
            Ypp = [Ypp_a, Ypp_b]
            Y = Ypp[0]
            U = sp.tile([128, 2, 2, J], F32, tag="U")
            TT = sp.tile([128, 2, 2, J], F32, tag="TT")
            TB = sp.tile([128, 2, 2, J], F32, tag="TB")
            AB = sp.tile([128, 2, 2, J], F32, tag="AB")
            for i in range(2):
                nc.vector.memset(Gpp[i][:], 0.0)
                nc.vector.memset(Ypp[i][:], 0.0)
            nc.vector.memset(Vpp[0][:], 0.0)
            nc.vector.memset(Vpp[1][:], 0.0)
            nc.vector.memset(U[:], 0.0)

            # G init: identity (or diag(p0) on core 0): G[p,e,:,W]
            nm0 = cp.tile([128, 1], F32, tag="nm0")
            nc.vector.tensor_scalar(nm0[:], isg[:], -1.0, 1.0, MULT, ADD)
            for e in range(2):
                nc.vector.scalar_tensor_tensor(
                    G[:, e, 0, W : W + 1], Ct[:, 16, e : e + 1], isg[:], nm0[:], MULT, ADD
                )
                nc.vector.tensor_scalar(
                    G[:, e, 1, W : W + 1], St[:, 16, e : e + 1], isg[:], None, MULT
                )


            # ---------------- phase A: 16 group steps ----------------
            # Per-step work is split DVE / ACT / Pool:
            #   DVE : V (2 ops), e=0 chain (T0/U0/TB0/AB0), Y0/Y1, blend e=0
            #   ACT : TB1
            #   Pool: e=1 chain (T1/U1/AB1), blend e=1
            # gm is folded into the last step's CR weights (WtL), gesc into
            # the blend, so the uniform blend is G' = gesc[e]*Y[e] + sgP[e].
            for s in range(nsteps):
                G = Gpp[s % 2]
                Gout = Gpp[(s + 1) % 2]
                Y = Ypp[s % 2]
                V = Vpp[s % 2]
                he = 2 * s + 1   # E-step half-width
                hc = 2 * s + 2   # CR half-width

                def sl(h, d=0):
                    return slice(W - h + d, W + h + 1 + d)

                cd1r = [d1r[:, s, e : e + 1] for e in range(2)]
                cd1i = [d1i[:, s, e : e + 1] for e in range(2)]
                cd2r = d2r[:, s, 0:1]
                cd2i = d2i[:, s, 0:1]

                # V = i*G on the CR window (reads beyond G's band are zeros)
                nc.vector.tensor_scalar(V[:, :, 0, sl(hc)], G[:, :, 1, sl(hc)], -1.0, None, MULT)
                nc.vector.tensor_copy(out=V[:, :, 1, sl(hc)], in_=G[:, :, 0, sl(hc)])

                # u[e] = d2 * G[1-e] (j-shifted -1 for e=0, +1 for e=1)
                nc.vector.tensor_scalar(TT[:, 0, :, sl(he)], V[:, 1, :, sl(he, -1)], cd2i, None, MULT)
                nc.vector.scalar_tensor_tensor(
                    U[:, 0, :, sl(he)], G[:, 1, :, sl(he, -1)], cd2r, TT[:, 0, :, sl(he)], MULT, ADD
                )
                nc.vector.tensor_scalar(TT[:, 1, :, sl(he)], V[:, 0, :, sl(he, +1)], cd2i, None, MULT)
                nc.vector.scalar_tensor_tensor(
                    U[:, 1, :, sl(he)], G[:, 0, :, sl(he, +1)], cd2r, TT[:, 1, :, sl(he)], MULT, ADD
                )

                # Y[e] = d1r[e]*G[e] + d1i[e]*V[e] + u[e], stored e-pre-shifted
                # (e=0 at j+1, e=1 at j-1) for aligned crossing-matmul reads
                for e in range(2):
                    sh = +1 if e == 0 else -1
                    nc.vector.scalar_tensor_tensor(
                        Y[:, e, :, sl(he, sh)], V[:, e, :, sl(he)], cd1i[e], U[:, e, :, sl(he)], MULT, ADD
                    )
                    nc.vector.scalar_tensor_tensor(
                        Y[:, e, :, sl(he, sh)], G[:, e, :, sl(he)], cd1r[e], Y[:, e, :, sl(he, sh)], MULT, ADD
                    )

                # CR: sgP[p,0] = (+-G2C)*Y[p-1,1] at j+1; sgP[p,1] = (+-G2C)*Y[p+1,0] at j-1
                lo8 = ((W - hc) // 8) * 8
                hi8 = ((W + hc + 1 + 7) // 8) * 8
                al = slice(lo8, hi8)
                Wts = WtL if s == nsteps - 1 else Wt
                sgP = pp.tile([128, 2, 2, J], F32, tag="sgP")
                nc.tensor.matmul(sgP[:, 1, 0, al], Wts[:, 2, :], Y[:, 0, 1, al], start=True, stop=True)
                nc.tensor.matmul(sgP[:, 1, 1, al], Wts[:, 3, :], Y[:, 0, 0, al], start=True, stop=True)
                nc.tensor.matmul(sgP[:, 0, 0, al], Wts[:, 0, :], Y[:, 1, 1, al], start=True, stop=True)
                nc.tensor.matmul(sgP[:, 0, 1, al], Wts[:, 1, :], Y[:, 1, 0, al], start=True, stop=True)

                # G' = gesc[e]*Y[e] + sgP[e]  (DVE; Pool cannot read PSUM)
                nc.vector.scalar_tensor_tensor(
                    Gout[:, 0, :, sl(hc)], Y[:, 0, :, sl(hc, +1)], gt[:, s, 0:1],
                    sgP[:, 0, :, sl(hc)], MULT, ADD,
                )
                nc.vector.scalar_tensor_tensor(
                    Gout[:, 1, :, sl(hc)], Y[:, 1, :, sl(hc, -1)], gt[:, s, 1:2],
                    sgP[:, 1, :, sl(hc)], MULT, ADD,
                )

            # ---------------- p129 post-scale (core 7; identity elsewhere) ---
            G = Gpp[nsteps % 2]
            fb = slice(W - 32, W + 33)
            for e in range(2):
                nc.vector.tensor_scalar(TB[:, 0, e, fb], G[:, e, 1, fb], seffN[:, e : e + 1], None, MULT)
                nc.vector.tensor_scalar(TB[:, 1, e, fb], G[:, e, 1, fb], ceff[:, e : e + 1], None, MULT)
                nc.vector.scalar_tensor_tensor(
                    G[:, e, 1, fb], G[:, e, 0, fb], seff[:, e : e + 1], TB[:, 1, e, fb], MULT, ADD
                )
                nc.vector.scalar_tensor_tensor(
                    G[:, e, 0, fb], G[:, e, 0, fb], ceff[:, e : e + 1], TB[:, 0, e, fb], MULT, ADD
                )


            # ---------------- pair -> linear row permutation ----------------
            pp2 = pp.tile([128, 2, 2, J], F32, tag="pp2", bufs=1)
            for rb in range(2):
                nc.tensor.matmul(pp2[:, rb, :, :], Pt[:, 2 * rb + 0, :], G[:, 0, :, :], start=True, stop=False)
                nc.tensor.matmul(pp2[:, rb, :, :], Pt[:, 2 * rb + 1, :], G[:, 1, :, :], start=False, stop=True)
            glin = sp.tile([128, 2, 2, JB], F16, tag="glin")
            nc.vector.tensor_copy(out=glin[:], in_=pp2[:, :, :, W - 32 : W + 33])

            # ---------------- stage out + AllGather + strip scatter ----------
            nc.sync.dma_start(
                gsend[:].rearrange("(rb p) ri j -> p rb ri j", rb=2, p=128),
                glin[:],
            )
            nc.gpsimd.collective_compute(
                "AllGather", mybir.AluOpType.bypass,
                replica_groups=[[0, 1, 2, 3, 4, 5, 6, 7]],
                ins=[gsend[:]],
                outs=[gall[:]],
            )
            ORDER = (7, 6, 5, 4, 3, 2, 1, 0)  # chain consumption order

            # ---------------- column-sharded F^T chain ------------------------
            # Core m computes F^T[:, 32m:32m+32] = G0^T ... G7^T @ SEL where
            # SEL is its one-hot column selector (per-core input data; the
            # program is uniform). The host assembles the full 256x256 output
            # from the 8 cores' 32-column slices. M is packed [p, ab, R|I]
            # (64 free) so one ltR matmul advances both planes at once.

            # keep PE clocked up through the collective: dummy matmuls on
            # stale data (PE is otherwise idle and would drop to the low
            # pstate, halving chain matmul speed)
            warm = pp.tile([128, 2, 2, J], F32, tag="sgP")
            for _w in range(WARMN):
                nc.tensor.matmul(
                    warm[:].rearrange('p a b c -> p (a b c)')[:, 0:256],
                    wrm[:, 0, :],
                    wrm[:].rearrange('p w f -> p (w f)'),
                    start=True, stop=True,
                )

            # per-k scatter + skew reads: one queue per k (keeps each k's
            # three DMAs in-order) and four queues round-robin so the first
            # four strips process fully in parallel right after the collective
            lts = [None] * 8
            ltIns = [None] * 8
            QS = (nc.sync, nc.scalar, nc.gpsimd)
            for idx, k in enumerate(ORDER):
                q = QS[idx % 3]
                q.dma_start(
                    strips[k, :, :, S0 + (W - 32) : S0 + (W - 32) + JB],
                    gall[k].rearrange("r pl j -> pl r j"),
                )
                ltk = lp.tile([128, 2, 2, 256], F16, tag=f"ltk{k}", bufs=1)
                for rb in range(2):
                    q.dma_start(ltk[:, rb, :, :], _skew_ap(strips, k, rb))
                lts[k] = ltk
                if idx > 0:  # k=7's imaginary-negated plane is never used
                    lik = lp.tile([128, 2, 256], F16, tag=f"ltIn{k}", bufs=1)
                    nc.gpsimd.tensor_scalar(lik[:], ltk[:, :, 1, :], -1.0, None, MULT)
                    ltIns[k] = lik

            # chain: M <- G_k^T @ M for k = 7..0, starting M = SEL (real)
            M = None
            for idx, k in enumerate(ORDER):
                lt = lts[k]
                lik = ltIns[k]
                P = pp.tile([128, 2, 64], F32, tag=f"P{idx % 2}", bufs=1)
                for ab in range(2):
                    abv = slice(128 * ab, 128 * (ab + 1))
                    if idx == 0:
                        # M = SEL: real one-hot; P.R = ltR^T SEL, P.I = ltI^T SEL
                        nc.tensor.matmul(P[:, ab, 0:32], lt[:, 0, 0, abv], selt[:, 0, :], start=True, stop=False)
                        nc.tensor.matmul(P[:, ab, 0:32], lt[:, 1, 0, abv], selt[:, 1, :], start=False, stop=False)
                        nc.tensor.matmul(P[:, ab, 32:64], lt[:, 0, 1, abv], selt[:, 0, :], start=False, stop=False, skip_group_check=True)
                        nc.tensor.matmul(P[:, ab, 32:64], lt[:, 1, 1, abv], selt[:, 1, :], start=False, stop=True, skip_group_check=True)
                    else:
                        # full-range ltR matmuls first (start zeroes the bank),
                        # then partial-range complex cross terms accumulate
                        nc.tensor.matmul(P[:, ab, :], lt[:, 0, 0, abv], M[:, 0, :], start=True, stop=False)
                        nc.tensor.matmul(P[:, ab, :], lt[:, 1, 0, abv], M[:, 1, :], start=False, stop=False)
                        nc.tensor.matmul(P[:, ab, 0:32], lik[:, 0, abv], M[:, 0, 32:64], start=False, stop=False, skip_group_check=True)
                        nc.tensor.matmul(P[:, ab, 0:32], lik[:, 1, abv], M[:, 1, 32:64], start=False, stop=False, skip_group_check=True)
                        nc.tensor.matmul(P[:, ab, 32:64], lt[:, 0, 1, abv], M[:, 0, 0:32], start=False, stop=False, skip_group_check=True)
                        nc.tensor.matmul(P[:, ab, 32:64], lt[:, 1, 1, abv], M[:, 1, 0:32], start=False, stop=True, skip_group_check=True)
                Mn = mp.tile([128, 2, 64], F16, tag=f"M{idx % 2}")
                nc.vector.tensor_copy(out=Mn[:, 0, :], in_=P[:, 0, :])
                nc.scalar.copy(Mn[:, 1, :], P[:, 1, :])
                M = Mn

            nc.sync.dma_start(out_d[:, 0, :], M[:, 0, :])
            nc.scalar.dma_start(out_d[:, 1, :], M[:, 1, :])


    return nc


def make_inputs(core: int, thetas: np.ndarray):
    m = core
    thg = np.concatenate(
        [thetas[16 * m + 1 : 16 * m + 17], thetas[0:1], thetas[129:130]], axis=0
    ).astype(np.float32)
    isg0 = np.full((128, 1), 1.0 if m == 0 else 0.0, np.float32)
    postm = np.full((128, 1), 1.0 if m == 7 else 0.0, np.float32)
    gesc = np.full((128, GITS, 2), G1S, np.float32)
    gesc[0, :, 0] = G2C
    gesc[127, :, 1] = G2C
    if m == 7:
        gesc[:, GITS - 1, :] = 1.0

    wdn = np.eye(128, k=1, dtype=np.float32)
    wup = np.eye(128, k=-1, dtype=np.float32)
    wconst = np.stack([-G2C * wdn, G2C * wdn, -G2C * wup, G2C * wup]).astype(np.float32)
    # last-step weights carry the gm mask (no trailing crossing on core 7)
    wconstL = np.zeros_like(wconst) if m == 7 else wconst

    permw = np.zeros((4, 128, 128), np.float32)
    for rb in range(2):
        for e in range(2):
            for p in range(64 * rb, 64 * rb + 64):
                permw[2 * rb + e, p, 2 * p + e - 128 * rb] = 1.0

    sel = np.zeros((128, 2, 32), np.float16)
    for j in range(32):
        r = 32 * m + j
        sel[r % 128, r // 128, j] = 1.0

    return {
        "thg": thg, "isg0": isg0, "postm": postm, "gescd": gesc,
        "wconst": wconst, "wconstL": wconstL, "permw": permw, "seld": sel,
    }


_CACHE = {}


def _get_nc():
    if "nc" not in _CACHE:
        nc = build_nc()
        fix_sync_waits(nc)
        _CACHE["nc"] = nc
    return _CACHE["nc"]


def _run(thetas: np.ndarray, trace: bool = False):
    thetas = np.ascontiguousarray(thetas, dtype=np.float32)
    assert thetas.shape == (130, N)
    nc = _get_nc()
    in_maps = [make_inputs(c, thetas) for c in range(NCORES)]
    res = run_bass_kernel_spmd(nc, in_maps, list(range(NCORES)), trace=trace)
    # core m's out [128, 2(ab), 64(R|I)] holds F^T[:, 32m:32m+32]
    FT = np.empty((256, 256), np.complex64)
    for m in range(NCORES):
        o = res.results[m]["out"]
        sl_ = (o[:, :, 0:32] + 1j * o[:, :, 32:64]).astype(np.complex64)
        FT[:, 32 * m : 32 * m + 32] = sl_.transpose(1, 0, 2).reshape(256, 32)
    return FT.T.copy(), res


def kernel(thetas: np.ndarray) -> np.ndarray:
    out, _ = _run(thetas, trace=False)
    return out

